# revision 26
# baseline (speedup 1.0000x reference)
"""VQ codebook assignment + nearest upsample on 8 NeuronCores.

Problem (per domain): given features f [B=4, C=256, H=64, W=128] and
centroids c [K=19, C=256], compute argmin_k ||f[b,:,h,w] - c_k||^2 and
nearest-upsample the [64,128] index map to [512,1024] (8x in each axis).
Two independent domains (cross-assigned centroids) x 4 batches = 8 cores,
one batch-image per core, no cross-core communication.

Per-core pipeline (v2 — PE-throughput-friendly K-partition matmuls):
  1. cross[k, px] via matmuls with the tiny centroid block stationary
     ([128, 19] per C-half) and 512-pixel feature chunks moving — full
     moving-side throughput instead of 19-column mini-matmuls.
  2. PSUM->SBUF copy on ScalarE folds scale/bias: Identity activation
     computes scores = 2*cross - ||c||^2 with a per-partition bias AP.
  3. PE transposes [19, 128] score chunks back to [128 px, 19] (pixel-
     partition layout) for the reduction.
  4. Argmax index on DVE: reduce_max over K, (is_ge * -1024 + iota)
     reduce_min, +1024 — first-match semantics, exact in f32 (argmax of
     2*cross - c2 == reference argmin of distance; the f^2 term is
     constant over k and dropped).
  5. DVE 32x32 block transpose + block-permute copies put the [w, h]
     index map into [h, w]; 8 broadcast copies replicate to
     [64, 8*1024] int32 and 8 SWDGE DMAs store 4KB-contiguous rows.

w is concatenated in front of the feature pixels (one tensor: the
block-0 chunk DMA covers both, so matmuls carry few semaphore waits).
Bacc.compile() legalizes any instruction with more than the 1-sync-wait
ISA limit.
"""

import numpy as np

import concourse.bass as bass
import concourse.mybir as mybir
import concourse.tile as tile
from concourse import bacc
from concourse.bass import ds
from concourse.bass_utils import run_bass_kernel_spmd
from concourse.masks import make_identity

F32 = mybir.dt.float32
I32 = mybir.dt.int32

B = 4
C = 256
H, W = 64, 128
K = 19
HL, WL = 512, 1024
NPIX = H * W          # 8192
RB = 8                # image rows per block
NB = H // RB          # 8 blocks
CH = 512              # matmul moving chunk (pixels)
RPC = CH // W         # image rows per chunk: 4
UP = HL // H          # 8x upsample
BIG = 1024.0
FWC = K + NPIX        # fw columns: [w | pixels]

_NC_CACHE = None


def _build_nc():
    nc = bacc.Bacc("TRN2", target_bir_lowering=False, debug=False)

    fw_in = nc.dram_tensor("fw", [C, FWC], F32, kind="ExternalInput")
    bias_in = nc.dram_tensor("bias", [128, K], F32, kind="ExternalInput")
    mask_out = nc.dram_tensor("mask", [HL, WL], I32, kind="ExternalOutput")

    fwv = fw_in.ap().rearrange("(a p) n -> a p n", a=2)       # [2, 128, FWC]
    outv = mask_out.ap().rearrange("(h y) x -> h y x", y=UP)  # [64, 8, 1024]

    with tile.TileContext(nc) as tc:
        with (
            tc.tile_pool(name="persist", bufs=1) as pp,
            tc.tile_pool(name="work", bufs=4) as wp,
            tc.tile_pool(name="psA", bufs=4, space="PSUM") as psA,
            tc.tile_pool(name="psB", bufs=3, space="PSUM") as psB,
            tc.tile_pool(name="psJ", bufs=1, space="PSUM") as psJ,
        ):
            fw0 = pp.tile([128, FWC], F32, tag="fw0")
            fw1 = pp.tile([128, FWC], F32, tag="fw1")
            bias128 = pp.tile([128, K], F32, tag="bias128")
            ident = pp.tile([K, K], F32, tag="ident")
            iota_i = pp.tile([128, K], I32, tag="iota_i")
            iotaf = pp.tile([128, K], F32, tag="iotaf")
            idxv = pp.tile([128, H], F32, tag="idxv")       # [w, h]
            tmp = pp.tile([128, H], F32, tag="tmp")         # block-transposed
            idxT = pp.tile([H, W], I32, tag="idxT")         # [h, w]
            rep = pp.tile([H, WL], I32, tag="rep")
            junk = pp.tile([128, CH], F32, tag="junk")

            # --- PE warm-up primer: ~14 junk matmuls keep the PE busy from
            # ~1.5us until the first feature chunk lands, so the HAM
            # activity window is warm (2.4 GHz) for every real matmul ---
            nc.gpsimd.memset(junk, 0.0)
            jps = psJ.tile([K, CH], F32, tag="jps")
            for _ in range(14):
                nc.tensor.matmul(
                    jps, junk[:, 0:K], junk[:, :], start=True, stop=True,
                )

            # --- setup ---
            nc.gpsimd.iota(iota_i, pattern=[[1, K]], base=0, channel_multiplier=0)
            nc.vector.tensor_copy(iotaf, iota_i)
            make_identity(nc, ident)
            nc.sync.dma_start(bias128, bias_in[:, :])

            # --- feature loads: block 0's chunk includes the w columns.
            # Triggers split across the two HWDGE engines (SP/ACT) so
            # trigger processing (~650ns each) runs in parallel. ---
            ld_slices = []
            for piece in range(4):
                lo = 0 if piece == 0 else K + piece * 256
                ld_slices.append(ds(lo, K + 256 - lo if piece == 0 else 256))
            for blk in range(1, NB):
                lo = K + blk * RB * W
                ld_slices.append(ds(lo, RB * W))
            for i, sl in enumerate(ld_slices):
                eng = nc.sync if i % 2 == 0 else nc.scalar
                eng.dma_start(fw0[:, sl], fwv[0, :, sl])
                eng.dma_start(fw1[:, sl], fwv[1, :, sl])

            iota_b = iotaf.rearrange("p (o k) -> p o k", o=1).to_broadcast(
                [128, RB, K]
            )

            # --- per-block: matmul -> scores -> transpose -> argmax index ---
            for blk in range(NB):
                ps2 = psB.tile([128, RB, K], F32, tag="ps2")
                for half in range(RB // RPC):
                    ch = blk * (RB // RPC) + half
                    colsl = ds(K + ch * CH, CH)
                    ps = psA.tile([K, CH], F32, tag="ps")
                    nc.tensor.matmul(
                        ps, fw0[:, 0:K], fw0[:, colsl],
                        start=True, stop=False,
                    )
                    nc.tensor.matmul(
                        ps, fw1[:, 0:K], fw1[:, colsl],
                        start=False, stop=True,
                    )
                    # plain Copy is bit-exact; the Identity-LUT bias/scale
                    # path has ~2^-12 relative error, enough to flip
                    # near-tie argmins
                    St = wp.tile([K, CH], F32, tag="St")
                    nc.scalar.copy(St, ps)
                    for r in range(RPC):
                        nc.tensor.transpose(
                            ps2[:, half * RPC + r],
                            St[:, ds(r * W, W)],
                            ident,
                        )
                # scores = cross - c2/2 (exact DVE add; ordering matches
                # the reference argmin of ||f-c||^2)
                S = wp.tile([128, RB, K], F32, tag="S")
                bias_b = bias128.rearrange("p (o k) -> p o k", o=1).to_broadcast(
                    [128, RB, K]
                )
                nc.vector.tensor_tensor(S, ps2, bias_b, op=mybir.AluOpType.add)
                maxv = wp.tile([128, RB], F32, tag="maxv")
                nc.vector.tensor_reduce(
                    maxv, S, axis=mybir.AxisListType.X, op=mybir.AluOpType.max
                )
                eq = wp.tile([128, RB, K], F32, tag="eq")
                maxv_b = maxv.rearrange("p (t o) -> p t o", o=1).to_broadcast(
                    [128, RB, K]
                )
                nc.vector.tensor_tensor(eq, S, maxv_b, op=mybir.AluOpType.is_ge)
                cand = wp.tile([128, RB, K], F32, tag="cand")
                nc.vector.scalar_tensor_tensor(
                    cand, eq, -BIG, iota_b,
                    op0=mybir.AluOpType.mult, op1=mybir.AluOpType.add,
                )
                nc.vector.tensor_reduce(
                    idxv[:, ds(blk * RB, RB)], cand,
                    axis=mybir.AxisListType.X, op=mybir.AluOpType.min,
                )

                # --- tail, overlapped: after each half of the blocks, emit
                # that h-half of the output (transpose, replicate, store) ---
                if blk % (NB // 2) != NB // 2 - 1:
                    continue
                hh = blk // (NB // 2)          # 0 or 1
                hsl = ds(hh * H // 2, H // 2)  # 32 h columns
                psl = ds(hh * 32, 32)          # matching partition rows
                nc.vector.tensor_scalar_add(idxv[:, hsl], idxv[:, hsl], BIG)
                nc.vector.transpose(tmp[:, hsl], idxv[:, hsl])
                for i in range(W // 32):
                    nc.vector.tensor_copy(
                        idxT[psl, ds(32 * i, 32)],
                        tmp[ds(32 * i, 32), hsl],
                    )
                # replicate 8x in x once on DVE; the 8x in y happens by
                # letting 8 store-DMAs re-read the same SBUF row (HWDGE,
                # 4KB-contiguous runs). GpSimd stays off SBUF — it shares
                # the DVE port pair and copies there stall both engines.
                idxT_b = idxT[psl].rearrange(
                    "p (w o) -> p w o", o=1
                ).to_broadcast([32, W, UP])
                nc.vector.tensor_copy(
                    rep[psl].rearrange("p (w x) -> p w x", w=W), idxT_b
                )
                for y in range(UP):
                    deng = nc.sync if y % 2 == 0 else nc.scalar
                    deng.dma_start(outv[psl, y], rep[psl])

    nc.compile()
    return nc


def _prep_domain(feature, centroid):
    """Per-core inputs for one domain: 4 batches against one centroid set."""
    c = np.ascontiguousarray(centroid, dtype=np.float32)
    w = c.T.astype(np.float32)                                  # [C, K]
    c2 = np.sum(c.astype(np.float32) ** 2, axis=1)              # [K]
    bias = np.ascontiguousarray(
        np.tile(-0.5 * c2[None, :], (128, 1)), dtype=np.float32
    )                                                           # [128, K]
    maps = []
    for b in range(B):
        f = np.asarray(feature[b], dtype=np.float32).reshape(C, NPIX)
        fw = np.ascontiguousarray(np.concatenate([w, f], axis=1))
        maps.append({"fw": fw, "bias": bias})
    return maps


def kernel(
    feature_s2t, feature_target, label_s2t, label_target,
    centroid_s2t, centroid_target,
):
    global _NC_CACHE
    if _NC_CACHE is None:
        _NC_CACHE = _build_nc()
    nc = _NC_CACHE

    # cross assignment: s2t features vs target centroids, and vice versa
    in_maps = _prep_domain(feature_s2t, centroid_target) + _prep_domain(
        feature_target, centroid_s2t
    )
    res = run_bass_kernel_spmd(nc, in_maps, core_ids=list(range(8))).results
    mask_s2t = np.stack([res[i]["mask"] for i in range(B)]).astype(np.int32)
    mask_target = np.stack([res[B + i]["mask"] for i in range(B)]).astype(
        np.int32
    )
    return (mask_s2t, mask_target)


# revision 27
# speedup vs baseline: 1.0659x; 1.0659x over previous
"""VQ codebook assignment + nearest upsample on 8 NeuronCores.

Problem (per domain): given features f [B=4, C=256, H=64, W=128] and
centroids c [K=19, C=256], compute argmin_k ||f[b,:,h,w] - c_k||^2 and
nearest-upsample the [64,128] index map to [512,1024] (8x in each axis).
Two independent domains (cross-assigned centroids) x 4 batches = 8 cores,
one batch-image per core, no cross-core communication.

Per-core pipeline (v2 — PE-throughput-friendly K-partition matmuls):
  1. cross[k, px] via matmuls with the tiny centroid block stationary
     ([128, 19] per C-half) and 512-pixel feature chunks moving — full
     moving-side throughput instead of 19-column mini-matmuls.
  2. PSUM->SBUF copy on ScalarE folds scale/bias: Identity activation
     computes scores = 2*cross - ||c||^2 with a per-partition bias AP.
  3. PE transposes [19, 128] score chunks back to [128 px, 19] (pixel-
     partition layout) for the reduction.
  4. Argmax index on DVE: reduce_max over K, (is_ge * -1024 + iota)
     reduce_min, +1024 — first-match semantics, exact in f32 (argmax of
     2*cross - c2 == reference argmin of distance; the f^2 term is
     constant over k and dropped).
  5. DVE 32x32 block transpose + block-permute copies put the [w, h]
     index map into [h, w]; 8 broadcast copies replicate to
     [64, 8*1024] int32 and 8 SWDGE DMAs store 4KB-contiguous rows.

w is concatenated in front of the feature pixels (one tensor: the
block-0 chunk DMA covers both, so matmuls carry few semaphore waits).
Bacc.compile() legalizes any instruction with more than the 1-sync-wait
ISA limit.
"""

import numpy as np

import concourse.bass as bass
import concourse.mybir as mybir
import concourse.tile as tile
from concourse import bacc
from concourse.bass import ds
from concourse.bass_utils import run_bass_kernel_spmd
from concourse.masks import make_identity

F32 = mybir.dt.float32
I32 = mybir.dt.int32

B = 4
C = 256
H, W = 64, 128
K = 19
HL, WL = 512, 1024
NPIX = H * W          # 8192
RB = 8                # image rows per block
NB = H // RB          # 8 blocks
CH = 512              # matmul moving chunk (pixels)
RPC = CH // W         # image rows per chunk: 4
UP = HL // H          # 8x upsample
BIG = 1024.0
FWC = K + NPIX        # fw columns: [w | pixels]

_NC_CACHE = None


def _build_nc():
    nc = bacc.Bacc("TRN2", target_bir_lowering=False, debug=False)

    fw_in = nc.dram_tensor("fw", [C, FWC], F32, kind="ExternalInput")
    bias_in = nc.dram_tensor("bias", [128, K], F32, kind="ExternalInput")
    mask_out = nc.dram_tensor("mask", [HL, WL], I32, kind="ExternalOutput")

    fwv = fw_in.ap().rearrange("(a p) n -> a p n", a=2)       # [2, 128, FWC]
    outv = mask_out.ap().rearrange("(h y) x -> h y x", y=UP)  # [64, 8, 1024]

    with tile.TileContext(nc) as tc:
        with (
            tc.tile_pool(name="persist", bufs=1) as pp,
            tc.tile_pool(name="work", bufs=4) as wp,
            tc.tile_pool(name="psA", bufs=4, space="PSUM") as psA,
            tc.tile_pool(name="psB", bufs=3, space="PSUM") as psB,
        ):
            fw0 = pp.tile([128, FWC], F32, tag="fw0")
            fw1 = pp.tile([128, FWC], F32, tag="fw1")
            bias128 = pp.tile([128, K], F32, tag="bias128")
            ident = pp.tile([K, K], F32, tag="ident")
            iota_i = pp.tile([128, K], I32, tag="iota_i")
            iotaf = pp.tile([128, K], F32, tag="iotaf")
            idxv = pp.tile([128, H], F32, tag="idxv")       # [w, h]
            tmp = pp.tile([128, H], F32, tag="tmp")         # block-transposed
            idxT = pp.tile([H, W], I32, tag="idxT")         # [h, w]
            rep = pp.tile([H, WL], I32, tag="rep")

            # --- setup ---
            nc.gpsimd.iota(iota_i, pattern=[[1, K]], base=0, channel_multiplier=0)
            nc.vector.tensor_copy(iotaf, iota_i)
            make_identity(nc, ident)
            nc.sync.dma_start(bias128, bias_in[:, :])

            # --- feature loads: block 0's chunk includes the w columns.
            # Triggers split across the two HWDGE engines (SP/ACT) so
            # trigger processing (~650ns each) runs in parallel. ---
            for blk in range(NB):
                lo = 0 if blk == 0 else K + blk * RB * W
                sl = ds(lo, K + RB * W - lo if blk == 0 else RB * W)
                eng = nc.sync if blk % 2 == 0 else nc.scalar
                eng.dma_start(fw0[:, sl], fwv[0, :, sl])
                eng.dma_start(fw1[:, sl], fwv[1, :, sl])

            iota_b = iotaf.rearrange("p (o k) -> p o k", o=1).to_broadcast(
                [128, RB, K]
            )

            # --- per-block: matmul -> scores -> transpose -> argmax index ---
            for blk in range(NB):
                ps2 = psB.tile([128, RB, K], F32, tag="ps2")
                for half in range(RB // RPC):
                    ch = blk * (RB // RPC) + half
                    colsl = ds(K + ch * CH, CH)
                    ps = psA.tile([K, CH], F32, tag="ps")
                    nc.tensor.matmul(
                        ps, fw0[:, 0:K], fw0[:, colsl],
                        start=True, stop=False,
                    )
                    nc.tensor.matmul(
                        ps, fw1[:, 0:K], fw1[:, colsl],
                        start=False, stop=True,
                    )
                    # plain Copy is bit-exact; the Identity-LUT bias/scale
                    # path has ~2^-12 relative error, enough to flip
                    # near-tie argmins
                    St = wp.tile([K, CH], F32, tag="St")
                    nc.scalar.copy(St, ps)
                    for r in range(RPC):
                        nc.tensor.transpose(
                            ps2[:, half * RPC + r],
                            St[:, ds(r * W, W)],
                            ident,
                        )
                # scores = cross - c2/2 (exact DVE add; ordering matches
                # the reference argmin of ||f-c||^2)
                S = wp.tile([128, RB, K], F32, tag="S")
                bias_b = bias128.rearrange("p (o k) -> p o k", o=1).to_broadcast(
                    [128, RB, K]
                )
                nc.vector.tensor_tensor(S, ps2, bias_b, op=mybir.AluOpType.add)
                maxv = wp.tile([128, RB], F32, tag="maxv")
                nc.vector.tensor_reduce(
                    maxv, S, axis=mybir.AxisListType.X, op=mybir.AluOpType.max
                )
                eq = wp.tile([128, RB, K], F32, tag="eq")
                maxv_b = maxv.rearrange("p (t o) -> p t o", o=1).to_broadcast(
                    [128, RB, K]
                )
                nc.vector.tensor_tensor(eq, S, maxv_b, op=mybir.AluOpType.is_ge)
                cand = wp.tile([128, RB, K], F32, tag="cand")
                nc.vector.scalar_tensor_tensor(
                    cand, eq, -BIG, iota_b,
                    op0=mybir.AluOpType.mult, op1=mybir.AluOpType.add,
                )
                nc.vector.tensor_reduce(
                    idxv[:, ds(blk * RB, RB)], cand,
                    axis=mybir.AxisListType.X, op=mybir.AluOpType.min,
                )

                # --- tail, overlapped: after each half of the blocks, emit
                # that h-half of the output (transpose, replicate, store) ---
                if blk % (NB // 2) != NB // 2 - 1:
                    continue
                hh = blk // (NB // 2)          # 0 or 1
                hsl = ds(hh * H // 2, H // 2)  # 32 h columns
                psl = ds(hh * 32, 32)          # matching partition rows
                nc.vector.tensor_scalar_add(idxv[:, hsl], idxv[:, hsl], BIG)
                nc.vector.transpose(tmp[:, hsl], idxv[:, hsl])
                for i in range(W // 32):
                    nc.vector.tensor_copy(
                        idxT[psl, ds(32 * i, 32)],
                        tmp[ds(32 * i, 32), hsl],
                    )
                # replicate 8x in x once on DVE; the 8x in y happens by
                # letting 8 store-DMAs re-read the same SBUF row (HWDGE,
                # 4KB-contiguous runs). GpSimd stays off SBUF — it shares
                # the DVE port pair and copies there stall both engines.
                idxT_b = idxT[psl].rearrange(
                    "p (w o) -> p w o", o=1
                ).to_broadcast([32, W, UP])
                nc.vector.tensor_copy(
                    rep[psl].rearrange("p (w x) -> p w x", w=W), idxT_b
                )
                for y in range(UP):
                    deng = nc.sync if y % 2 == 0 else nc.scalar
                    deng.dma_start(outv[psl, y], rep[psl])

    nc.compile()
    return nc


def _prep_domain(feature, centroid):
    """Per-core inputs for one domain: 4 batches against one centroid set."""
    c = np.ascontiguousarray(centroid, dtype=np.float32)
    w = c.T.astype(np.float32)                                  # [C, K]
    c2 = np.sum(c.astype(np.float32) ** 2, axis=1)              # [K]
    bias = np.ascontiguousarray(
        np.tile(-0.5 * c2[None, :], (128, 1)), dtype=np.float32
    )                                                           # [128, K]
    maps = []
    for b in range(B):
        f = np.asarray(feature[b], dtype=np.float32).reshape(C, NPIX)
        fw = np.ascontiguousarray(np.concatenate([w, f], axis=1))
        maps.append({"fw": fw, "bias": bias})
    return maps


def kernel(
    feature_s2t, feature_target, label_s2t, label_target,
    centroid_s2t, centroid_target,
):
    global _NC_CACHE
    if _NC_CACHE is None:
        _NC_CACHE = _build_nc()
    nc = _NC_CACHE

    # cross assignment: s2t features vs target centroids, and vice versa
    in_maps = _prep_domain(feature_s2t, centroid_target) + _prep_domain(
        feature_target, centroid_s2t
    )
    res = run_bass_kernel_spmd(nc, in_maps, core_ids=list(range(8))).results
    mask_s2t = np.stack([res[i]["mask"] for i in range(B)]).astype(np.int32)
    mask_target = np.stack([res[B + i]["mask"] for i in range(B)]).astype(
        np.int32
    )
    return (mask_s2t, mask_target)


# revision 28
# speedup vs baseline: 1.0768x; 1.0102x over previous
"""VQ codebook assignment + nearest upsample on 8 NeuronCores.

Problem (per domain): given features f [B=4, C=256, H=64, W=128] and
centroids c [K=19, C=256], compute argmin_k ||f[b,:,h,w] - c_k||^2 and
nearest-upsample the [64,128] index map to [512,1024] (8x in each axis).
Two independent domains (cross-assigned centroids) x 4 batches = 8 cores,
one batch-image per core, no cross-core communication.

Per-core pipeline (v2 — PE-throughput-friendly K-partition matmuls):
  1. cross[k, px] via matmuls with the tiny centroid block stationary
     ([128, 19] per C-half) and 512-pixel feature chunks moving — full
     moving-side throughput instead of 19-column mini-matmuls.
  2. PSUM->SBUF copy on ScalarE folds scale/bias: Identity activation
     computes scores = 2*cross - ||c||^2 with a per-partition bias AP.
  3. PE transposes [19, 128] score chunks back to [128 px, 19] (pixel-
     partition layout) for the reduction.
  4. Argmax index on DVE: reduce_max over K, (is_ge * -1024 + iota)
     reduce_min, +1024 — first-match semantics, exact in f32 (argmax of
     2*cross - c2 == reference argmin of distance; the f^2 term is
     constant over k and dropped).
  5. DVE 32x32 block transpose + block-permute copies put the [w, h]
     index map into [h, w]; 8 broadcast copies replicate to
     [64, 8*1024] int32 and 8 SWDGE DMAs store 4KB-contiguous rows.

w is concatenated in front of the feature pixels (one tensor: the
block-0 chunk DMA covers both, so matmuls carry few semaphore waits).
Bacc.compile() legalizes any instruction with more than the 1-sync-wait
ISA limit.
"""

import numpy as np

import concourse.bass as bass
import concourse.mybir as mybir
import concourse.tile as tile
from concourse import bacc
from concourse.bass import ds
from concourse.bass_utils import run_bass_kernel_spmd
from concourse.masks import make_identity

F32 = mybir.dt.float32
I32 = mybir.dt.int32

B = 4
C = 256
H, W = 64, 128
K = 19
HL, WL = 512, 1024
NPIX = H * W          # 8192
RB = 8                # image rows per block
NB = H // RB          # 8 blocks
CH = 512              # matmul moving chunk (pixels)
RPC = CH // W         # image rows per chunk: 4
UP = HL // H          # 8x upsample
BIG = 1024.0
FWC = K + NPIX        # fw columns: [w | pixels]

_NC_CACHE = None


def _build_nc():
    nc = bacc.Bacc("TRN2", target_bir_lowering=False, debug=False)

    fw_in = nc.dram_tensor("fw", [C, FWC], F32, kind="ExternalInput")
    bias_in = nc.dram_tensor("bias", [128, K], F32, kind="ExternalInput")
    mask_out = nc.dram_tensor("mask", [HL, WL], I32, kind="ExternalOutput")

    fwv = fw_in.ap().rearrange("(a p) n -> a p n", a=2)       # [2, 128, FWC]
    outv = mask_out.ap().rearrange("(h y) x -> h y x", y=UP)  # [64, 8, 1024]

    with tile.TileContext(nc) as tc:
        with (
            tc.tile_pool(name="persist", bufs=1) as pp,
            tc.tile_pool(name="work", bufs=6) as wp,
            tc.tile_pool(name="psA", bufs=5, space="PSUM") as psA,
            tc.tile_pool(name="psB", bufs=3, space="PSUM") as psB,
        ):
            fw0 = pp.tile([128, FWC], F32, tag="fw0")
            fw1 = pp.tile([128, FWC], F32, tag="fw1")
            bias128 = pp.tile([128, K], F32, tag="bias128")
            ident = pp.tile([K, K], F32, tag="ident")
            iota_i = pp.tile([128, K], I32, tag="iota_i")
            iotaf = pp.tile([128, K], F32, tag="iotaf")
            idxv = pp.tile([128, H], F32, tag="idxv")       # [w, h]
            tmp = pp.tile([128, H], F32, tag="tmp")         # block-transposed
            idxT = pp.tile([H, W], I32, tag="idxT")         # [h, w]
            rep = pp.tile([H, WL], I32, tag="rep")

            # --- setup ---
            nc.gpsimd.iota(iota_i, pattern=[[1, K]], base=0, channel_multiplier=0)
            nc.vector.tensor_copy(iotaf, iota_i)
            make_identity(nc, ident)
            nc.sync.dma_start(bias128, bias_in[:, :])

            # --- feature loads: block 0's chunk includes the w columns.
            # Triggers split across the two HWDGE engines (SP/ACT) so
            # trigger processing (~650ns each) runs in parallel. ---
            for blk in range(NB):
                lo = 0 if blk == 0 else K + blk * RB * W
                sl = ds(lo, K + RB * W - lo if blk == 0 else RB * W)
                eng = nc.sync if blk % 2 == 0 else nc.scalar
                eng.dma_start(fw0[:, sl], fwv[0, :, sl])
                eng.dma_start(fw1[:, sl], fwv[1, :, sl])

            iota_b = iotaf.rearrange("p (o k) -> p o k", o=1).to_broadcast(
                [128, RB, K]
            )

            # --- per-block: matmul -> scores -> transpose -> argmax index ---
            for blk in range(NB):
                ps2 = psB.tile([128, RB, K], F32, tag="ps2")
                for half in range(RB // RPC):
                    ch = blk * (RB // RPC) + half
                    colsl = ds(K + ch * CH, CH)
                    ps = psA.tile([K, CH], F32, tag="ps")
                    nc.tensor.matmul(
                        ps, fw0[:, 0:K], fw0[:, colsl],
                        start=True, stop=False,
                    )
                    nc.tensor.matmul(
                        ps, fw1[:, 0:K], fw1[:, colsl],
                        start=False, stop=True,
                    )
                    # plain Copy is bit-exact; the Identity-LUT bias/scale
                    # path has ~2^-12 relative error, enough to flip
                    # near-tie argmins
                    St = wp.tile([K, CH], F32, tag="St")
                    nc.scalar.copy(St, ps)
                    for r in range(RPC):
                        nc.tensor.transpose(
                            ps2[:, half * RPC + r],
                            St[:, ds(r * W, W)],
                            ident,
                        )
                # scores = cross - c2/2 (exact DVE add; ordering matches
                # the reference argmin of ||f-c||^2)
                S = wp.tile([128, RB, K], F32, tag="S")
                bias_b = bias128.rearrange("p (o k) -> p o k", o=1).to_broadcast(
                    [128, RB, K]
                )
                nc.vector.tensor_tensor(S, ps2, bias_b, op=mybir.AluOpType.add)
                maxv = wp.tile([128, RB], F32, tag="maxv")
                nc.vector.tensor_reduce(
                    maxv, S, axis=mybir.AxisListType.X, op=mybir.AluOpType.max
                )
                eq = wp.tile([128, RB, K], F32, tag="eq")
                maxv_b = maxv.rearrange("p (t o) -> p t o", o=1).to_broadcast(
                    [128, RB, K]
                )
                nc.vector.tensor_tensor(eq, S, maxv_b, op=mybir.AluOpType.is_ge)
                cand = wp.tile([128, RB, K], F32, tag="cand")
                nc.vector.scalar_tensor_tensor(
                    cand, eq, -BIG, iota_b,
                    op0=mybir.AluOpType.mult, op1=mybir.AluOpType.add,
                )
                nc.vector.tensor_reduce(
                    idxv[:, ds(blk * RB, RB)], cand,
                    axis=mybir.AxisListType.X, op=mybir.AluOpType.min,
                )

                # --- tail, overlapped: after each half of the blocks, emit
                # that h-half of the output (transpose, replicate, store) ---
                if blk % (NB // 2) != NB // 2 - 1:
                    continue
                hh = blk // (NB // 2)          # 0 or 1
                hsl = ds(hh * H // 2, H // 2)  # 32 h columns
                psl = ds(hh * 32, 32)          # matching partition rows
                nc.vector.tensor_scalar_add(idxv[:, hsl], idxv[:, hsl], BIG)
                nc.vector.transpose(tmp[:, hsl], idxv[:, hsl])
                for i in range(W // 32):
                    nc.vector.tensor_copy(
                        idxT[psl, ds(32 * i, 32)],
                        tmp[ds(32 * i, 32), hsl],
                    )
                # replicate 8x in x once on DVE; the 8x in y happens by
                # letting 8 store-DMAs re-read the same SBUF row (HWDGE,
                # 4KB-contiguous runs). GpSimd stays off SBUF — it shares
                # the DVE port pair and copies there stall both engines.
                idxT_b = idxT[psl].rearrange(
                    "p (w o) -> p w o", o=1
                ).to_broadcast([32, W, UP])
                nc.vector.tensor_copy(
                    rep[psl].rearrange("p (w x) -> p w x", w=W), idxT_b
                )
                for y in range(UP):
                    deng = nc.sync if y % 2 == 0 else nc.scalar
                    deng.dma_start(outv[psl, y], rep[psl])

    nc.compile()
    return nc


def _prep_domain(feature, centroid):
    """Per-core inputs for one domain: 4 batches against one centroid set."""
    c = np.ascontiguousarray(centroid, dtype=np.float32)
    w = c.T.astype(np.float32)                                  # [C, K]
    c2 = np.sum(c.astype(np.float32) ** 2, axis=1)              # [K]
    bias = np.ascontiguousarray(
        np.tile(-0.5 * c2[None, :], (128, 1)), dtype=np.float32
    )                                                           # [128, K]
    maps = []
    for b in range(B):
        f = np.asarray(feature[b], dtype=np.float32).reshape(C, NPIX)
        fw = np.ascontiguousarray(np.concatenate([w, f], axis=1))
        maps.append({"fw": fw, "bias": bias})
    return maps


def kernel(
    feature_s2t, feature_target, label_s2t, label_target,
    centroid_s2t, centroid_target,
):
    global _NC_CACHE
    if _NC_CACHE is None:
        _NC_CACHE = _build_nc()
    nc = _NC_CACHE

    # cross assignment: s2t features vs target centroids, and vice versa
    in_maps = _prep_domain(feature_s2t, centroid_target) + _prep_domain(
        feature_target, centroid_s2t
    )
    res = run_bass_kernel_spmd(nc, in_maps, core_ids=list(range(8))).results
    mask_s2t = np.stack([res[i]["mask"] for i in range(B)]).astype(np.int32)
    mask_target = np.stack([res[B + i]["mask"] for i in range(B)]).astype(
        np.int32
    )
    return (mask_s2t, mask_target)
